# revision 3
# baseline (speedup 1.0000x reference)
"""Trainium2 Bass kernel for DeepBayesianDKVMN (nn_DeepBayesianDKVMN_39857296507058).

Math restructuring
------------------
The reference's sequential Bayesian-write scan is *linear* in the memory
state: the per-step precision/evidence increments depend only on step-t
inputs, never on the evolving state.  So the scan collapses to exclusive
cumulative sums over time, and everything else is batch-parallel:

  - front end: q_table[q_ids] @ q2k_w @ key_embeds.T folds into ONE
    [NQ+1, M] "sim table" gather (host precomputes the table, device does
    a dma_gather of 256B rows).
  - softmax: logits = sim + bias[t,m]; exp(bias) is multiplied in as eb.
  - per-(b,t) evidence vector comb[b,t,:] is a rank-4 combination of four
    fixed V-vectors, so the [S,M,V] write-aggregation reduces to six
    [S,M] batch sums:  colsum, P (precision), H_k (k=0..3).
  - the 1/Z softmax normalizations fold into the per-(b,t) feature matrix
    F' = F * [1/Z, 1/Z, 1/Z^2 x4], contracted against [ee | ee^2] by tiny
    per-t PE matmuls (contraction over the 128 batch rows on partitions).
  - AllReduce over the six [S,M] stats; exclusive cumsums via
    tensor_tensor_scan; read vector g[t,m]; preds = zr * (ee . g) + pred_b.

Performance structure (v2)
--------------------------
  - the dma_gather descriptor stream is spread over all 4 SWDGE queues
    (2 sub-gathers x 16 chunks, round-robin) so four SDMA engines drain
    concurrently; single-queue drain was the old 512us bottleneck.
  - 16 chunks of 32 timesteps pipeline gather/exp/softmax/PE stats.
  - eef chunks are DMA-transposed into a resident [2t x 64m, b] tensor as
    they are produced (hidden under the gather), so the final read-dot
    runs from SBUF with 2-col LDWEIGHTS + 128-col matmuls.
  - the [S,M] stat AllReduce is split in two so the first 3/4 overlaps
    the tail of the gather phase.

Sharding: batch 1024 -> 128 rows per core across 8 cores (data parallel),
as the sharding hint suggests; the all-reduce is the per-slot aggregated
evidence/precision, shrunk by the rank-4 trick.
"""

import numpy as np
from contextlib import ExitStack

import concourse.bass as bass
import concourse.tile as tile
from concourse import bacc, mybir
from concourse.bass_utils import run_bass_kernel_spmd

# problem dims (hardcoded per spec)
B, S, M, K, V, E, NQ, C = 1024, 512, 64, 64, 128, 64, 10000, 4
NCORES = 8
BL = B // NCORES            # 128 batch rows per core
TC = 32                     # timestep chunk
NCH = S // TC               # 16 chunks
NIDX = BL * TC              # gather indices per chunk = 4096
SPLIT = 2                   # sub-gathers per chunk (queue spreading)
NQUEUES = 4                 # SWDGE queues (max 4)
ARSPLIT = 12                # chunks in the first (overlapped) AllReduce
F32 = mybir.dt.float32
F16 = mybir.dt.float16
I16 = mybir.dt.int16
GSCALE = 1024.0            # fp16 pre-scale for the tiny g values
ALU = mybir.AluOpType
AXT = mybir.AxisListType
ACTF = mybir.ActivationFunctionType

_CACHE = {}


def _build(single_core=False):
    nc = bacc.Bacc("TRN2", target_bir_lowering=False, debug=False,
                   num_devices=1 if single_core else NCORES,
                   num_swdge_queues=NQUEUES)

    t_simtab = nc.dram_tensor("simtab", [NQ + 1, M], F32, kind="ExternalInput")
    t_idx = nc.dram_tensor("idx16", [128, NCH * NIDX // 16], I16,
                           kind="ExternalInput")
    t_ftab = nc.dram_tensor("ftab", [BL, S, 6], F16, kind="ExternalInput")
    t_ebr = nc.dram_tensor("ebr", [BL, S * M], F16, kind="ExternalInput")
    t_sc = nc.dram_tensor("scal", [M, 8], F32, kind="ExternalInput")
    t_pb = nc.dram_tensor("pb", [BL, 1], F32, kind="ExternalInput")
    t_preds = nc.dram_tensor("preds", [BL, S], F32, kind="ExternalOutput")

    with tile.TileContext(nc) as tc:
        _build_body(nc, tc, single_core, t_simtab, t_idx, t_ftab, t_ebr,
                    t_sc, t_pb, t_preds)
    nc.compile()
    return nc


def _build_body(nc, tc, single_core, t_simtab, t_idx, t_ftab, t_ebr,
                t_sc, t_pb, t_preds):
    with ExitStack() as ctx:
        cpool = ctx.enter_context(tc.tile_pool(name="const", bufs=1))
        dpool = ctx.enter_context(tc.tile_pool(name="dram", bufs=1,
                                               space="DRAM"))
        # internal DRAM
        d_hinA = dpool.tile([M, ARSPLIT * TC * 6], F32)
        d_hinB = dpool.tile([M, (NCH - ARSPLIT) * TC * 6], F32)
        d_houtA = dpool.tile([M, ARSPLIT * TC * 6], F32, addr_space="Shared")
        d_houtB = dpool.tile([M, (NCH - ARSPLIT) * TC * 6], F32,
                             addr_space="Shared")
        d_g16 = dpool.tile([M, S], F16)

        # resident SBUF
        idx_sb = cpool.tile([128, NCH * NIDX // 16], I16)
        nc.sync.dma_start(idx_sb[:], t_idx.ap())
        ftab_sb = cpool.tile([BL, S, 6], F16)
        nc.sync.dma_start(ftab_sb[:], t_ftab.ap())
        sc_sb = cpool.tile([M, 8], F32)
        nc.sync.dma_start(sc_sb[:], t_sc.ap())
        pb_sb = cpool.tile([BL, 1], F32)
        nc.sync.dma_start(pb_sb[:], t_pb.ap())
        zr = cpool.tile([BL, S], F32)
        eet = cpool.tile([128, S // 2, 128], F16)  # (t%2)*64+m, t//2, b

        # ---------------- phase A: per-chunk softmax stats + H matmuls
        apool = ctx.enter_context(tc.tile_pool(name="pha", bufs=3))
        bpool = ctx.enter_context(tc.tile_pool(name="phb", bufs=2))
        epool = ctx.enter_context(tc.tile_pool(name="phe", bufs=3))
        spool = ctx.enter_context(tc.tile_pool(name="phs", bufs=2))
        pspool = ctx.enter_context(
            tc.tile_pool(name="php", bufs=2, space="PSUM"))
        qctr = 0
        for c in range(NCH):
            ts = slice(c * TC, (c + 1) * TC)
            ge = apool.tile([BL, TC * M], F32, tag="ge")
            ge3 = ge[:].rearrange("p (a b) -> p a b", b=M)
            nsub = NIDX // SPLIT
            tsub = TC // SPLIT
            for a in range(SPLIT):
                i0 = c * NIDX // 16 + a * nsub // 16
                nc.gpsimd.dma_gather(
                    out_ap=ge3[:, a * tsub:(a + 1) * tsub, :],
                    in_ap=t_simtab.ap(),
                    idxs_ap=idx_sb[:, i0:i0 + nsub // 16],
                    num_idxs=nsub,
                    num_idxs_reg=nsub,
                    elem_size=M,
                    single_packet=False,
                    queue_num=qctr % NQUEUES,
                )
                qctr += 1
            eb = bpool.tile([BL, TC * M], F16, tag="eb")
            nc.sync.dma_start(
                eb[:], t_ebr.ap()[:, c * TC * M:(c + 1) * TC * M])
            # exp in place, then ee = exp(sim) * eb cast to fp16
            nc.scalar.activation(ge[:], ge[:], ACTF.Exp)
            eef = epool.tile([BL, TC * M], F16, tag="eef")
            nc.vector.tensor_tensor(eef[:], ge[:], eb[:], ALU.mult)
            ee3 = eef[:].rearrange("p (a b) -> p a b", b=M)
            z = spool.tile([BL, TC], F32, tag="z")
            nc.vector.tensor_reduce(z[:], ee3, axis=AXT.X, op=ALU.add)
            zrc = zr[:, ts]
            nc.vector.reciprocal(zrc, z[:])
            zr2 = spool.tile([BL, TC], F32, tag="zr2")
            nc.vector.tensor_tensor(zr2[:], zrc, zrc, ALU.mult)
            fp = spool.tile([BL, TC, 6], F16, tag="fp")
            nc.vector.tensor_tensor(
                fp[:, :, 0:2], ftab_sb[:, ts, 0:2],
                zrc.unsqueeze(2).broadcast_to([BL, TC, 2]), ALU.mult)
            nc.vector.tensor_tensor(
                fp[:, :, 2:6], ftab_sb[:, ts, 2:6],
                zr2[:].unsqueeze(2).broadcast_to([BL, TC, 4]), ALU.mult)
            e2 = bpool.tile([BL, TC * M], F16, tag="e2")
            nc.scalar.activation(e2[:], eef[:], ACTF.Square)
            e23 = e2[:].rearrange("p (a b) -> p a b", b=M)
            hp = pspool.tile([M, TC * 6], F32, tag="hp")
            for t in range(TC):
                nc.tensor.matmul(hp[:, t * 6:t * 6 + 2], lhsT=ee3[:, t, :],
                                 rhs=fp[:, t, 0:2], start=True, stop=True)
                nc.tensor.matmul(hp[:, t * 6 + 2:t * 6 + 6],
                                 lhsT=e23[:, t, :], rhs=fp[:, t, 2:6],
                                 start=True, stop=True)
            hbc = spool.tile([M, TC * 6], F32, tag="hbc")
            nc.vector.tensor_copy(hbc[:], hp[:])
            if c < ARSPLIT:
                nc.scalar.dma_start(
                    d_hinA[:, c * TC * 6:(c + 1) * TC * 6], hbc[:])
            else:
                cc0 = c - ARSPLIT
                nc.scalar.dma_start(
                    d_hinB[:, cc0 * TC * 6:(cc0 + 1) * TC * 6], hbc[:])
            # transpose eef into the resident read-dot operand
            nc.sync.dma_start_transpose(
                eet[:, c * TC // 2:(c + 1) * TC // 2, :], eef[:])
            if c == ARSPLIT - 1:
                if single_core:
                    nc.sync.dma_start(d_houtA[:], d_hinA[:])
                else:
                    nc.gpsimd.collective_compute(
                        "AllReduce", ALU.add,
                        replica_groups=[list(range(NCORES))],
                        ins=[d_hinA[:].opt()],
                        outs=[d_houtA[:].opt()],
                    )
            if c == NCH - 1:
                if single_core:
                    nc.sync.dma_start(d_houtB[:], d_hinB[:])
                else:
                    nc.gpsimd.collective_compute(
                        "AllReduce", ALU.add,
                        replica_groups=[list(range(NCORES))],
                        ins=[d_hinB[:].opt()],
                        outs=[d_houtB[:].opt()],
                    )

        hs = cpool.tile([M, S, 6], F32)
        nc.sync.dma_start(
            hs[:, 0:ARSPLIT * TC, :],
            d_houtA[:].rearrange("m (s k) -> m s k", k=6))
        nc.sync.dma_start(
            hs[:, ARSPLIT * TC:S, :],
            d_houtB[:].rearrange("m (s k) -> m s k", k=6))

        # ---------------- phase C: cumsums + read vector g  (all [M, S])
        cs_v = hs[:, :, 0]
        p_v = hs[:, :, 1]
        cpool2 = ctx.enter_context(tc.tile_pool(name="phc", bufs=1))
        css = cpool2.tile([M, S], F32)
        nc.vector.tensor_scalar_add(css[:], cs_v, 1e-8)
        rcs = cpool2.tile([M, S], F32)
        nc.vector.reciprocal(rcs[:], css[:])
        cc = cpool2.tile([M, S], F32)
        nc.vector.tensor_tensor(cc[:], p_v, cs_v, ALU.mult)
        nc.vector.tensor_tensor(cc[:], cc[:], rcs[:], ALU.mult)
        nc.vector.tensor_scalar(cc[:], cc[:], 1.0 / B, None, ALU.mult)
        sfac = cpool2.tile([M, S], F32)
        nc.vector.tensor_tensor(sfac[:], cc[:], rcs[:], ALU.mult)

        num = cpool2.tile([M, S], F32)
        ch = cpool2.tile([M, S + 1], F32, tag="chk")
        hsk = cpool2.tile([M, S], F32, tag="hsk")
        for k in range(4):
            nc.vector.tensor_tensor(hsk[:], hs[:, :, 2 + k], sfac[:], ALU.mult)
            nc.vector.memset(ch[:, 0:1], 0.0)
            nc.vector.tensor_tensor_scan(ch[:, 1:S + 1], hsk[:], hsk[:], 0.0,
                                         ALU.add, ALU.bypass)
            if k == 0:
                nc.vector.tensor_scalar(num[:], ch[:, 0:S], sc_sb[:, 0:1],
                                        None, ALU.mult)
            else:
                nc.vector.scalar_tensor_tensor(num[:], ch[:, 0:S],
                                               sc_sb[:, k:k + 1], num[:],
                                               ALU.mult, ALU.add)
            ch = cpool2.tile([M, S + 1], F32, tag="chk")
            hsk = cpool2.tile([M, S], F32, tag="hsk")
        # num += n0pw ; den = alo + CC_excl ; g = num / den
        nc.vector.tensor_scalar_add(num[:], num[:], sc_sb[:, 5:6])
        ccs = cpool2.tile([M, S + 1], F32)
        nc.vector.memset(ccs[:, 0:1], 0.0)
        nc.vector.tensor_tensor_scan(ccs[:, 1:S + 1], cc[:], cc[:], 0.0,
                                     ALU.add, ALU.bypass)
        den = cpool2.tile([M, S], F32)
        nc.vector.tensor_scalar_add(den[:], ccs[:, 0:S], sc_sb[:, 4:5])
        rden = cpool2.tile([M, S], F32)
        nc.vector.reciprocal(rden[:], den[:])
        g = cpool2.tile([M, S], F32)
        nc.vector.tensor_tensor(g[:], num[:], rden[:], ALU.mult)
        # g scaled into the fp16 normal range, bounced via DRAM so it can
        # be loaded onto both partition halves (even/odd t block-columns)
        g16 = cpool2.tile([M, S], F16)
        nc.vector.tensor_scalar(g16[:], g[:], GSCALE, None, ALU.mult)
        nc.sync.dma_start(d_g16[:], g16[:])
        gdup = cpool.tile([128, S], F16)
        nc.sync.dma_start(gdup[0:M, :], d_g16[:])
        nc.sync.dma_start(gdup[M:128, :], d_g16[:])
        # block-diagonal column pairs: col 2j keeps only the even-t (top)
        # half, col 2j+1 only the odd-t (bottom) half
        gblk = cpool.tile([128, S], F16)
        nc.vector.memset(gblk[:], 0.0)
        gd2 = gdup[:].rearrange("p (j two) -> p j two", two=2)
        gb2 = gblk[:].rearrange("p (j two) -> p j two", two=2)
        nc.vector.tensor_copy(gb2[0:M, :, 0], gd2[0:M, :, 0])
        nc.vector.tensor_copy(gb2[M:128, :, 1], gd2[M:128, :, 1])

        # ---------------- phase D: dot[b, 2j:2j+2] = eet_pair.T @ gblk_pair
        # (the block-diagonal gblk columns keep even/odd t separate), landing
        # directly in [b, t] layout; preds = zr/GSCALE * dot + pred_b
        rtile = cpool.tile([BL, S], F32)
        dpool2 = ctx.enter_context(tc.tile_pool(name="phd", bufs=2))
        psd = ctx.enter_context(
            tc.tile_pool(name="phdp", bufs=2, space="PSUM"))
        NG = S // 128  # 4 groups of 128 timesteps
        for gi in range(NG):
            psD = psd.tile([BL, 128], F32, tag="psD")
            for jj in range(64):
                j = gi * 64 + jj
                nc.tensor.matmul(psD[:, 2 * jj:2 * jj + 2],
                                 lhsT=eet[:, j, :],
                                 rhs=gblk[:, 2 * j:2 * j + 2],
                                 start=True, stop=True)
            gsl = slice(gi * 128, (gi + 1) * 128)
            rt32 = dpool2.tile([BL, 128], F32, tag="rt32")
            nc.vector.tensor_tensor(rt32[:], psD[:], zr[:, gsl], ALU.mult)
            nc.vector.tensor_scalar(rtile[:, gsl], rt32[:], 1.0 / GSCALE,
                                    pb_sb[:, 0:1], ALU.mult, ALU.add)
        nc.sync.dma_start(t_preds.ap(), rtile[:])


def _softplus(x):
    return np.logaddexp(0.0, x)


def _host_prep(inputs):
    """All the cheap host-side precomputation; returns per-core in_maps."""
    q_ids = np.asarray(inputs["q_ids"], np.int64)          # [B, S]
    responses = np.asarray(inputs["responses"], np.int64)  # [B, S]
    q_table = np.asarray(inputs["q_table"], np.float32)
    key_embeds = np.asarray(inputs["key_embeds"], np.float32)
    alpha_mean = np.asarray(inputs["alpha_mean"], np.float32)
    alpha_log_var = np.asarray(inputs["alpha_log_var"], np.float32)
    beta_base = np.asarray(inputs["beta_base"], np.float32)
    beta_offsets = np.asarray(inputs["beta_offsets"], np.float32)
    theta_mean0 = np.asarray(inputs["theta_mean0"], np.float32)
    theta_log_var0 = np.asarray(inputs["theta_log_var0"], np.float32)
    q2k_w = np.asarray(inputs["q2k_w"], np.float32)
    q2k_b = np.asarray(inputs["q2k_b"], np.float32)
    qa_w = np.asarray(inputs["qa_w"], np.float32)
    qa_b = np.asarray(inputs["qa_b"], np.float32)
    qae_w = np.asarray(inputs["qae_w"], np.float32)
    qae_b = np.asarray(inputs["qae_b"], np.float32)
    pred_w = np.asarray(inputs["pred_w"], np.float32)
    pred_b = np.asarray(inputs["pred_b"], np.float32)
    alpha_noise = np.asarray(inputs["alpha_noise"], np.float32)
    beta_noise = np.asarray(inputs["beta_noise"], np.float32)

    # sim table: folds q_table @ q2k_w @ key_embeds.T (+ bias) into a gather
    w_qm = q2k_w @ key_embeds.T                            # [E, M]
    b_m = q2k_b @ key_embeds.T                             # [M]
    simtab = (q_table @ w_qm + b_m[None]).astype(np.float32)

    # per-(t, m) logit bias -> eb = exp(bias)
    alpha = np.exp(alpha_mean[None] + alpha_noise
                   * np.exp(0.5 * alpha_log_var)[None])    # [S, M]
    base = beta_base[None] + beta_noise * 0.1              # [S, M]
    offs = _softplus(beta_offsets)                         # [M, C-1]
    cum = np.concatenate([np.zeros((M, 1), np.float32),
                          np.cumsum(offs, 1)[:, :C - 2]], 1)
    beta_mean = base + cum.mean(1)[None]
    diff_sim = np.exp(-0.5 * beta_mean ** 2)
    ebt = np.exp(0.3 * alpha + 0.2 * diff_sim).astype(np.float32)  # [S, M]
    ebrep = np.ascontiguousarray(
        np.broadcast_to(ebt.reshape(1, S * M).astype(np.float16),
                        (BL, S * M)))

    # evidence scalars per (b, t)
    rn = responses.astype(np.float32) / (C - 1)
    p = np.clip(rn, 0.01, 0.99)
    ae = np.log(p) - np.log1p(-p)
    pr = 0.5 + np.abs(rn - 0.5) * 2.0
    q01 = q_ids.astype(np.float32) / NQ

    # rank-4 decomposition of comb over V
    w0v = qa_w[0] @ qae_w
    w1v = qa_w[1] @ qae_w
    bv = qa_b @ qae_w + qae_b
    pw = pred_w[:, 0]
    gp = 0.5 * np.array([w0v @ pw, w1v @ pw, bv @ pw, pw.sum()], np.float32)

    alo = np.exp(-theta_log_var0[:, 0])                    # [M]
    n0pw = alo * (theta_mean0 @ pw)                        # [M]
    sc = np.zeros((M, 8), np.float32)
    sc[:, 0:4] = gp[None, :]
    sc[:, 4] = alo
    sc[:, 5] = n0pw

    pb = np.full((BL, 1), float(pred_b[0]), np.float32)

    in_maps = []
    for core in range(NCORES):
        bs = slice(core * BL, (core + 1) * BL)
        qs = q_ids[bs]                                     # [128, S]
        # gather indices, chunk-major, wrapped in 16 partitions
        blocks = []
        for c in range(NCH):
            flat = qs[:, c * TC:(c + 1) * TC].T.reshape(-1)  # t-major
            w16 = flat.reshape(NIDX // 16, 16).T             # [16, NIDX/16]
            blocks.append(np.tile(w16, (8, 1)))
        idx16 = np.concatenate(blocks, axis=1).astype(np.int16)

        ftab = np.empty((BL, S, 6), np.float16)
        ftab[:, :, 0] = 1.0
        ftab[:, :, 1] = pr[bs]
        ftab[:, :, 2] = q01[bs]
        ftab[:, :, 3] = rn[bs]
        ftab[:, :, 4] = 1.0
        ftab[:, :, 5] = ae[bs]

        in_maps.append({
            "simtab": simtab,
            "idx16": idx16,
            "ftab": ftab,
            "ebr": ebrep,
            "scal": sc,
            "pb": pb,
        })
    return in_maps


def _run(in_maps, **kw):
    if "nc" not in _CACHE:
        _CACHE["nc"] = _build()
    res = run_bass_kernel_spmd(_CACHE["nc"], in_maps,
                               core_ids=list(range(NCORES)), **kw)
    preds = np.concatenate([res.results[c]["preds"] for c in range(NCORES)],
                           axis=0)
    return preds.astype(np.float32), res


def kernel(**inputs) -> np.ndarray:
    return _run(_host_prep(inputs))[0]


if __name__ == "__main__":
    pass


# revision 12
# speedup vs baseline: 1.9264x; 1.9264x over previous
"""Trainium2 Bass kernel for DeepBayesianDKVMN (nn_DeepBayesianDKVMN_39857296507058).

Math restructuring
------------------
The reference's sequential Bayesian-write scan is *linear* in the memory
state: the per-step precision/evidence increments depend only on step-t
inputs, never on the evolving state.  So the scan collapses to exclusive
cumulative sums over time, and everything else is batch-parallel:

  - front end: q_table[q_ids] @ q2k_w @ key_embeds.T folds into ONE
    [NQ+1, M] "sim table" gather (host precomputes the table, device does
    a dma_gather of 256B rows).
  - softmax: logits = sim + bias[t,m]; exp(bias) is multiplied in as eb.
  - per-(b,t) evidence vector comb[b,t,:] is a rank-4 combination of four
    fixed V-vectors, so the [S,M,V] write-aggregation reduces to six
    [S,M] batch sums:  colsum, P (precision), H_k (k=0..3).
  - the 1/Z softmax normalizations fold into the per-(b,t) feature matrix
    F' = F * [1/Z, 1/Z, 1/Z^2 x4], contracted against [ee | ee^2] by tiny
    per-t PE matmuls (contraction over the 128 batch rows on partitions).
  - AllReduce over the six [S,M] stats; exclusive cumsums via
    tensor_tensor_scan; read vector g[t,m]; preds = zr * (ee . g) + pred_b.

Performance structure (v2)
--------------------------
  - the dma_gather descriptor stream is spread over all 4 SWDGE queues
    (2 sub-gathers x 16 chunks, round-robin) so four SDMA engines drain
    concurrently; single-queue drain was the old 512us bottleneck.
  - 16 chunks of 32 timesteps pipeline gather/exp/softmax/PE stats.
  - eef chunks are DMA-transposed into a resident [2t x 64m, b] tensor as
    they are produced (hidden under the gather), so the final read-dot
    runs from SBUF with 2-col LDWEIGHTS + 128-col matmuls.
  - the [S,M] stat AllReduce is split in two so the first 3/4 overlaps
    the tail of the gather phase.

Sharding: batch 1024 -> 128 rows per core across 8 cores (data parallel),
as the sharding hint suggests; the all-reduce is the per-slot aggregated
evidence/precision, shrunk by the rank-4 trick.
"""

import numpy as np
from contextlib import ExitStack

import concourse.bass as bass
import concourse.tile as tile
from concourse import bacc, mybir
from concourse.bass_utils import run_bass_kernel_spmd

# problem dims (hardcoded per spec)
B, S, M, K, V, E, NQ, C = 1024, 512, 64, 64, 128, 64, 10000, 4
NCORES = 8
BL = B // NCORES            # 128 batch rows per core
TC = 32                     # timestep chunk
NCH = S // TC               # 16 chunks
NIDX = BL * TC              # gather indices per chunk = 4096
SPLIT = 2                   # sub-gathers per chunk (queue spreading)
NQUEUES = 4                 # SWDGE queues (max 4)
ARSPLIT = 12                # chunks in the first (overlapped) AllReduce
F32 = mybir.dt.float32
F16 = mybir.dt.float16
I16 = mybir.dt.int16
GSCALE = 1024.0            # fp16 pre-scale for the tiny g values
ALU = mybir.AluOpType
AXT = mybir.AxisListType
ACTF = mybir.ActivationFunctionType

_CACHE = {}


def _build(single_core=False):
    nc = bacc.Bacc("TRN2", target_bir_lowering=False, debug=False,
                   num_devices=1 if single_core else NCORES,
                   num_swdge_queues=NQUEUES)

    t_simtab = nc.dram_tensor("simtab", [NQ + 1, M], F32, kind="ExternalInput")
    t_idx = nc.dram_tensor("idx16", [128, NCH * NIDX // 16], I16,
                           kind="ExternalInput")
    t_ftab = nc.dram_tensor("ftab", [BL, S, 6], F16, kind="ExternalInput")
    t_ebr = nc.dram_tensor("ebr", [BL, S * M], F16, kind="ExternalInput")
    t_sc = nc.dram_tensor("scal", [M, 8], F32, kind="ExternalInput")
    t_pb = nc.dram_tensor("pb", [BL, 1], F32, kind="ExternalInput")
    t_ident = nc.dram_tensor("ident", [128, 128], F16, kind="ExternalInput")
    t_preds = nc.dram_tensor("preds", [BL, S], F32, kind="ExternalOutput")

    with tile.TileContext(nc) as tc:
        _build_body(nc, tc, single_core, t_simtab, t_idx, t_ftab, t_ebr,
                    t_sc, t_pb, t_ident, t_preds)
    nc.compile()
    return nc


def _build_body(nc, tc, single_core, t_simtab, t_idx, t_ftab, t_ebr,
                t_sc, t_pb, t_ident, t_preds):
    with ExitStack() as ctx:
        cpool = ctx.enter_context(tc.tile_pool(name="const", bufs=1))
        dpool = ctx.enter_context(tc.tile_pool(name="dram", bufs=1,
                                               space="DRAM"))
        # internal DRAM
        d_hinA = dpool.tile([M, ARSPLIT * TC * 6], F32)
        d_hinB = dpool.tile([M, (NCH - ARSPLIT) * TC * 6], F32)
        d_houtA = dpool.tile([M, ARSPLIT * TC * 6], F32, addr_space="Shared")
        d_houtB = dpool.tile([M, (NCH - ARSPLIT) * TC * 6], F32,
                             addr_space="Shared")
        d_g16 = dpool.tile([M, S], F16)

        # resident SBUF
        idx_sb = cpool.tile([128, NCH * NIDX // 16], I16)
        nc.sync.dma_start(idx_sb[:], t_idx.ap())
        ftab_sb = cpool.tile([BL, S, 6], F16)
        nc.sync.dma_start(ftab_sb[:], t_ftab.ap())
        sc_sb = cpool.tile([M, 8], F32)
        nc.sync.dma_start(sc_sb[:], t_sc.ap())
        pb_sb = cpool.tile([BL, 1], F32)
        nc.sync.dma_start(pb_sb[:], t_pb.ap())
        ident_sb = cpool.tile([128, 128], F16)
        nc.sync.dma_start(ident_sb[:], t_ident.ap())
        zr = cpool.tile([BL, S], F32)
        eet = cpool.tile([128, S // 2, 128], F16)  # (t%2)*64+m, t//2, b

        # ---------------- phase A: per-chunk softmax stats + H matmuls
        actx = ctx.enter_context(ExitStack())
        apool = actx.enter_context(tc.tile_pool(name="pha", bufs=3))
        bpool = actx.enter_context(tc.tile_pool(name="phb", bufs=2))
        epool = actx.enter_context(tc.tile_pool(name="phe", bufs=3))
        spool = actx.enter_context(tc.tile_pool(name="phs", bufs=2))
        pspool = actx.enter_context(
            tc.tile_pool(name="php", bufs=2, space="PSUM"))
        tpool = actx.enter_context(
            tc.tile_pool(name="pht", bufs=2, space="PSUM"))
        qctr = 0
        for c in range(NCH):
            ts = slice(c * TC, (c + 1) * TC)
            ge = apool.tile([BL, TC * M], F32, tag="ge")
            ge3 = ge[:].rearrange("p (a b) -> p a b", b=M)
            nsub = NIDX // SPLIT
            tsub = TC // SPLIT
            for a in range(SPLIT):
                i0 = c * NIDX // 16 + a * nsub // 16
                nc.gpsimd.dma_gather(
                    out_ap=ge3[:, a * tsub:(a + 1) * tsub, :],
                    in_ap=t_simtab.ap(),
                    idxs_ap=idx_sb[:, i0:i0 + nsub // 16],
                    num_idxs=nsub,
                    num_idxs_reg=nsub,
                    elem_size=M,
                    single_packet=False,
                    queue_num=qctr % NQUEUES,
                )
                qctr += 1
            eb = bpool.tile([BL, TC * M], F16, tag="eb")
            nc.sync.dma_start(
                eb[:], t_ebr.ap()[:, c * TC * M:(c + 1) * TC * M])
            # exp in place, then ee = exp(sim) * eb cast to fp16
            nc.scalar.activation(ge[:], ge[:], ACTF.Exp)
            eef = epool.tile([BL, TC * M], F16, tag="eef")
            nc.vector.tensor_tensor(eef[:], ge[:], eb[:], ALU.mult)
            ee3 = eef[:].rearrange("p (a b) -> p a b", b=M)
            z = spool.tile([BL, TC], F32, tag="z")
            nc.vector.tensor_reduce(z[:], ee3, axis=AXT.X, op=ALU.add)
            zrc = zr[:, ts]
            nc.vector.reciprocal(zrc, z[:])
            zr2 = spool.tile([BL, TC], F32, tag="zr2")
            nc.vector.tensor_tensor(zr2[:], zrc, zrc, ALU.mult)
            fp = spool.tile([BL, TC, 6], F16, tag="fp")
            nc.vector.tensor_tensor(
                fp[:, :, 0:2], ftab_sb[:, ts, 0:2],
                zrc.unsqueeze(2).broadcast_to([BL, TC, 2]), ALU.mult)
            nc.vector.tensor_tensor(
                fp[:, :, 2:6], ftab_sb[:, ts, 2:6],
                zr2[:].unsqueeze(2).broadcast_to([BL, TC, 4]), ALU.mult)
            e2 = bpool.tile([BL, TC * M], F16, tag="e2")
            nc.scalar.activation(e2[:], eef[:], ACTF.Square)
            e23 = e2[:].rearrange("p (a b) -> p a b", b=M)
            hp = pspool.tile([M, TC * 6], F32, tag="hp")
            for t in range(TC):
                nc.tensor.matmul(hp[:, t * 6:t * 6 + 2], lhsT=ee3[:, t, :],
                                 rhs=fp[:, t, 0:2], start=True, stop=True)
                nc.tensor.matmul(hp[:, t * 6 + 2:t * 6 + 6],
                                 lhsT=e23[:, t, :], rhs=fp[:, t, 2:6],
                                 start=True, stop=True)
            hbc = spool.tile([M, TC * 6], F32, tag="hbc")
            nc.vector.tensor_copy(hbc[:], hp[:])
            if c < ARSPLIT:
                nc.scalar.dma_start(
                    d_hinA[:, c * TC * 6:(c + 1) * TC * 6], hbc[:])
            else:
                cc0 = c - ARSPLIT
                nc.scalar.dma_start(
                    d_hinB[:, cc0 * TC * 6:(cc0 + 1) * TC * 6], hbc[:])
            # transpose eef into the resident read-dot operand via the PE
            # (dma_start_transpose starves the gather SDMA queues), then
            # drain PSUM->SBUF on the scalar engine to keep the DVE light
            pst = tpool.tile([128, TC // 2, 128], F16, tag="pst")
            eeb = eef[:].rearrange("p (k b) -> p k b", b=128)
            for kk in range(TC // 2):
                nc.tensor.transpose(pst[:, kk, :], eeb[:, kk, :], ident_sb[:])
            nc.scalar.activation(
                eet[:, c * TC // 2:(c + 1) * TC // 2, :], pst[:], ACTF.Copy)
            if c == ARSPLIT - 1:
                if single_core:
                    nc.sync.dma_start(d_houtA[:], d_hinA[:])
                else:
                    nc.gpsimd.collective_compute(
                        "AllReduce", ALU.add,
                        replica_groups=[list(range(NCORES))],
                        ins=[d_hinA[:].opt()],
                        outs=[d_houtA[:].opt()],
                    )
            if c == NCH - 1:
                if single_core:
                    nc.sync.dma_start(d_houtB[:], d_hinB[:])
                else:
                    nc.gpsimd.collective_compute(
                        "AllReduce", ALU.add,
                        replica_groups=[list(range(NCORES))],
                        ins=[d_hinB[:].opt()],
                        outs=[d_houtB[:].opt()],
                    )
        actx.close()

        hs = cpool.tile([M, S, 6], F32)
        nc.sync.dma_start(
            hs[:, 0:ARSPLIT * TC, :],
            d_houtA[:].rearrange("m (s k) -> m s k", k=6))
        nc.sync.dma_start(
            hs[:, ARSPLIT * TC:S, :],
            d_houtB[:].rearrange("m (s k) -> m s k", k=6))

        # ---------------- phase C: cumsums + read vector g  (all [M, S])
        cs_v = hs[:, :, 0]
        p_v = hs[:, :, 1]
        cpool2 = ctx.enter_context(tc.tile_pool(name="phc", bufs=1))
        css = cpool2.tile([M, S], F32)
        nc.vector.tensor_scalar_add(css[:], cs_v, 1e-8)
        rcs = cpool2.tile([M, S], F32)
        nc.vector.reciprocal(rcs[:], css[:])
        cc = cpool2.tile([M, S], F32)
        nc.vector.tensor_tensor(cc[:], p_v, cs_v, ALU.mult)
        nc.vector.tensor_tensor(cc[:], cc[:], rcs[:], ALU.mult)
        nc.vector.tensor_scalar(cc[:], cc[:], 1.0 / B, None, ALU.mult)
        sfac = cpool2.tile([M, S], F32)
        nc.vector.tensor_tensor(sfac[:], cc[:], rcs[:], ALU.mult)

        num = cpool2.tile([M, S], F32)
        ch = cpool2.tile([M, S + 1], F32, tag="chk")
        hsk = cpool2.tile([M, S], F32, tag="hsk")
        for k in range(4):
            nc.vector.tensor_tensor(hsk[:], hs[:, :, 2 + k], sfac[:], ALU.mult)
            nc.vector.memset(ch[:, 0:1], 0.0)
            nc.vector.tensor_tensor_scan(ch[:, 1:S + 1], hsk[:], hsk[:], 0.0,
                                         ALU.add, ALU.bypass)
            if k == 0:
                nc.vector.tensor_scalar(num[:], ch[:, 0:S], sc_sb[:, 0:1],
                                        None, ALU.mult)
            else:
                nc.vector.scalar_tensor_tensor(num[:], ch[:, 0:S],
                                               sc_sb[:, k:k + 1], num[:],
                                               ALU.mult, ALU.add)
            ch = cpool2.tile([M, S + 1], F32, tag="chk")
            hsk = cpool2.tile([M, S], F32, tag="hsk")
        # num += n0pw ; den = alo + CC_excl ; g = num / den
        nc.vector.tensor_scalar_add(num[:], num[:], sc_sb[:, 5:6])
        ccs = cpool2.tile([M, S + 1], F32)
        nc.vector.memset(ccs[:, 0:1], 0.0)
        nc.vector.tensor_tensor_scan(ccs[:, 1:S + 1], cc[:], cc[:], 0.0,
                                     ALU.add, ALU.bypass)
        den = cpool2.tile([M, S], F32)
        nc.vector.tensor_scalar_add(den[:], ccs[:, 0:S], sc_sb[:, 4:5])
        rden = cpool2.tile([M, S], F32)
        nc.vector.reciprocal(rden[:], den[:])
        g = cpool2.tile([M, S], F32)
        nc.vector.tensor_tensor(g[:], num[:], rden[:], ALU.mult)
        # g scaled into the fp16 normal range, bounced via DRAM so it can
        # be loaded onto both partition halves (even/odd t block-columns)
        g16 = cpool2.tile([M, S], F16)
        nc.vector.tensor_scalar(g16[:], g[:], GSCALE, None, ALU.mult)
        nc.sync.dma_start(d_g16[:], g16[:])
        gdup = cpool.tile([128, S], F16)
        nc.sync.dma_start(gdup[0:M, :], d_g16[:])
        nc.sync.dma_start(gdup[M:128, :], d_g16[:])
        # block-diagonal column pairs: col 2j keeps only the even-t (top)
        # half, col 2j+1 only the odd-t (bottom) half
        gblk = cpool.tile([128, S], F16)
        nc.vector.memset(gblk[:], 0.0)
        gd2 = gdup[:].rearrange("p (j two) -> p j two", two=2)
        gb2 = gblk[:].rearrange("p (j two) -> p j two", two=2)
        nc.vector.tensor_copy(gb2[0:M, :, 0], gd2[0:M, :, 0])
        nc.vector.tensor_copy(gb2[M:128, :, 1], gd2[M:128, :, 1])

        # ---------------- phase D: dot[b, 2j:2j+2] = eet_pair.T @ gblk_pair
        # (the block-diagonal gblk columns keep even/odd t separate), landing
        # directly in [b, t] layout; preds = zr/GSCALE * dot + pred_b
        rtile = cpool.tile([BL, S], F32)
        dpool2 = ctx.enter_context(tc.tile_pool(name="phd", bufs=2))
        psd = ctx.enter_context(
            tc.tile_pool(name="phdp", bufs=2, space="PSUM"))
        NG = S // 128  # 4 groups of 128 timesteps
        for gi in range(NG):
            psD = psd.tile([BL, 128], F32, tag="psD")
            for jj in range(64):
                j = gi * 64 + jj
                nc.tensor.matmul(psD[:, 2 * jj:2 * jj + 2],
                                 lhsT=eet[:, j, :],
                                 rhs=gblk[:, 2 * j:2 * j + 2],
                                 start=True, stop=True)
            gsl = slice(gi * 128, (gi + 1) * 128)
            rt32 = dpool2.tile([BL, 128], F32, tag="rt32")
            nc.vector.tensor_tensor(rt32[:], psD[:], zr[:, gsl], ALU.mult)
            nc.vector.tensor_scalar(rtile[:, gsl], rt32[:], 1.0 / GSCALE,
                                    pb_sb[:, 0:1], ALU.mult, ALU.add)
        nc.sync.dma_start(t_preds.ap(), rtile[:])


def _softplus(x):
    return np.logaddexp(0.0, x)


def _host_prep(inputs):
    """All the cheap host-side precomputation; returns per-core in_maps."""
    q_ids = np.asarray(inputs["q_ids"], np.int64)          # [B, S]
    responses = np.asarray(inputs["responses"], np.int64)  # [B, S]
    q_table = np.asarray(inputs["q_table"], np.float32)
    key_embeds = np.asarray(inputs["key_embeds"], np.float32)
    alpha_mean = np.asarray(inputs["alpha_mean"], np.float32)
    alpha_log_var = np.asarray(inputs["alpha_log_var"], np.float32)
    beta_base = np.asarray(inputs["beta_base"], np.float32)
    beta_offsets = np.asarray(inputs["beta_offsets"], np.float32)
    theta_mean0 = np.asarray(inputs["theta_mean0"], np.float32)
    theta_log_var0 = np.asarray(inputs["theta_log_var0"], np.float32)
    q2k_w = np.asarray(inputs["q2k_w"], np.float32)
    q2k_b = np.asarray(inputs["q2k_b"], np.float32)
    qa_w = np.asarray(inputs["qa_w"], np.float32)
    qa_b = np.asarray(inputs["qa_b"], np.float32)
    qae_w = np.asarray(inputs["qae_w"], np.float32)
    qae_b = np.asarray(inputs["qae_b"], np.float32)
    pred_w = np.asarray(inputs["pred_w"], np.float32)
    pred_b = np.asarray(inputs["pred_b"], np.float32)
    alpha_noise = np.asarray(inputs["alpha_noise"], np.float32)
    beta_noise = np.asarray(inputs["beta_noise"], np.float32)

    # sim table: folds q_table @ q2k_w @ key_embeds.T (+ bias) into a gather
    w_qm = q2k_w @ key_embeds.T                            # [E, M]
    b_m = q2k_b @ key_embeds.T                             # [M]
    simtab = (q_table @ w_qm + b_m[None]).astype(np.float32)

    # per-(t, m) logit bias -> eb = exp(bias)
    alpha = np.exp(alpha_mean[None] + alpha_noise
                   * np.exp(0.5 * alpha_log_var)[None])    # [S, M]
    base = beta_base[None] + beta_noise * 0.1              # [S, M]
    offs = _softplus(beta_offsets)                         # [M, C-1]
    cum = np.concatenate([np.zeros((M, 1), np.float32),
                          np.cumsum(offs, 1)[:, :C - 2]], 1)
    beta_mean = base + cum.mean(1)[None]
    diff_sim = np.exp(-0.5 * beta_mean ** 2)
    ebt = np.exp(0.3 * alpha + 0.2 * diff_sim).astype(np.float32)  # [S, M]
    ebrep = np.ascontiguousarray(
        np.broadcast_to(ebt.reshape(1, S * M).astype(np.float16),
                        (BL, S * M)))

    # evidence scalars per (b, t)
    rn = responses.astype(np.float32) / (C - 1)
    p = np.clip(rn, 0.01, 0.99)
    ae = np.log(p) - np.log1p(-p)
    pr = 0.5 + np.abs(rn - 0.5) * 2.0
    q01 = q_ids.astype(np.float32) / NQ

    # rank-4 decomposition of comb over V
    w0v = qa_w[0] @ qae_w
    w1v = qa_w[1] @ qae_w
    bv = qa_b @ qae_w + qae_b
    pw = pred_w[:, 0]
    gp = 0.5 * np.array([w0v @ pw, w1v @ pw, bv @ pw, pw.sum()], np.float32)

    alo = np.exp(-theta_log_var0[:, 0])                    # [M]
    n0pw = alo * (theta_mean0 @ pw)                        # [M]
    sc = np.zeros((M, 8), np.float32)
    sc[:, 0:4] = gp[None, :]
    sc[:, 4] = alo
    sc[:, 5] = n0pw

    pb = np.full((BL, 1), float(pred_b[0]), np.float32)
    ident = np.eye(128, dtype=np.float16)

    in_maps = []
    for core in range(NCORES):
        bs = slice(core * BL, (core + 1) * BL)
        qs = q_ids[bs]                                     # [128, S]
        # gather indices, chunk-major, wrapped in 16 partitions
        blocks = []
        for c in range(NCH):
            flat = qs[:, c * TC:(c + 1) * TC].T.reshape(-1)  # t-major
            w16 = flat.reshape(NIDX // 16, 16).T             # [16, NIDX/16]
            blocks.append(np.tile(w16, (8, 1)))
        idx16 = np.concatenate(blocks, axis=1).astype(np.int16)

        ftab = np.empty((BL, S, 6), np.float16)
        ftab[:, :, 0] = 1.0
        ftab[:, :, 1] = pr[bs]
        ftab[:, :, 2] = q01[bs]
        ftab[:, :, 3] = rn[bs]
        ftab[:, :, 4] = 1.0
        ftab[:, :, 5] = ae[bs]

        in_maps.append({
            "simtab": simtab,
            "idx16": idx16,
            "ftab": ftab,
            "ebr": ebrep,
            "scal": sc,
            "pb": pb,
            "ident": ident,
        })
    return in_maps


def _run(in_maps, **kw):
    if "nc" not in _CACHE:
        _CACHE["nc"] = _build()
    res = run_bass_kernel_spmd(_CACHE["nc"], in_maps,
                               core_ids=list(range(NCORES)), **kw)
    preds = np.concatenate([res.results[c]["preds"] for c in range(NCORES)],
                           axis=0)
    return preds.astype(np.float32), res


def kernel(**inputs) -> np.ndarray:
    return _run(_host_prep(inputs))[0]


if __name__ == "__main__":
    pass
